# revision 1
# baseline (speedup 1.0000x reference)
"""Trainium2 Bass kernel for a causal dense-transformer attention layer.

Reference computation (b=4, s=2048, d=1024, 16 heads, dh=64):
  qkv = x0 @ W_in ; causal softmax attention ; out = attn @ W_o
  y = LayerNorm(out + x0)   (no affine, eps=1e-5)

Sharding over 8 cores: core = (batch bi = core//2, head-group tp = core%2).
Each core computes QKV projection + attention for its 8 heads of one batch
(tensor parallel over head groups), then an AllToAll within the (bi) pair
re-shards from (heads-half, full seq) to (all heads, seq-half) so the output
projection + residual + LayerNorm run fully local, with no all-reduce.

On-chip layout: scores are computed transposed (keys on partitions, queries
on the free axis) so attn @ V needs no transposes; the softmax denominator
comes from an extra ones-column matmul packed into spare PE column groups.
Projections run in float32r (full PE rate at N=512), attention and output
projection in fp16 with fp32 PSUM accumulation. Softmax skips the max
subtraction (logits are O(5) here, exp is safe in fp32/fp16 range), and the
causal mask is applied multiplicatively after exp, which is exact.
"""

import os
import sys
from contextlib import ExitStack

import numpy as np

for _p in ("/opt/trn_rl_repo",):
    if os.path.isdir(_p) and _p not in sys.path:
        sys.path.insert(0, _p)

import concourse.bass as bass
import concourse.tile as tile
from concourse import bacc
from concourse import mybir
from concourse.bass_utils import run_bass_kernel_spmd

B, S, D = 4, 2048, 1024
NH, DH = 16, 64
HL = NH // 2          # heads per core
SH = S // 2           # output seq rows per core
NCORES = 8
SCALE = DH ** -0.5    # 0.125
LN_EPS = 1e-5

F32R = mybir.dt.float32r
F16 = mybir.dt.float16
F32 = mybir.dt.float32
Exp = mybir.ActivationFunctionType.Exp
Sqrt = mybir.ActivationFunctionType.Sqrt


def build_nc():
    nc = bacc.Bacc("TRN2", target_bir_lowering=False, num_devices=NCORES)
    xT = nc.declare_dram_parameter("xT", [D, S], F16, isOutput=False)
    wqk = nc.declare_dram_parameter("wqk", [D, 2 * HL * DH], F16, isOutput=False)
    wv = nc.declare_dram_parameter("wv", [D, HL * DH], F16, isOutput=False)
    wo = nc.declare_dram_parameter("wo", [HL * DH, D], F16, isOutput=False)
    xres = nc.declare_dram_parameter("xres", [SH, D], F32, isOutput=False)
    cmsk = nc.declare_dram_parameter("cmask", [128, 4 * 512], F16, isOutput=False)
    out = nc.declare_dram_parameter("out", [SH, D], F32, isOutput=True)

    with tile.TileContext(nc, num_cores=NCORES) as tc, ExitStack() as top:
        persist = top.enter_context(tc.tile_pool(name="persist", bufs=1))
        # QT rows 0..511 (tiles 0-3, head pair t on tile t), KT rows 512..1023
        qkt = [persist.tile([128, S], F16, name=f"qkt{m}") for m in range(8)]
        # V in (seq-part, head*dh free) orientation, 16 seq tiles
        vsb = [persist.tile([128, HL * (DH + 1)], F16, name=f"vsb{m}") for m in range(16)]
        # attn-out^T (head*dh on partitions, seq free)
        aot = [persist.tile([128, S], F16, name=f"aot{t}") for t in range(4)]
        # unnormalized attn-out^T (fp32) + softmax denominators, normalized
        # in a deferred pass so the PSUM banks free up immediately
        aot_u = [persist.tile([128, S], F32, name=f"aotu{t}") for t in range(4)]
        cm = persist.tile([128, 4 * 512], F16, name="cm")
        ones = persist.tile([128, 1], F16, name="ones")
        eps_t = persist.tile([128, 1], F32, name="eps_t")
        nc.vector.memset(ones, 1.0)
        nc.vector.memset(eps_t, LN_EPS)
        for m in range(16):
            vones = vsb[m].rearrange("p (h c) -> p h c", c=DH + 1)[:, :, DH:DH + 1]
            nc.vector.memset(vones, 1.0)
        nc.sync.dma_start(out=cm, in_=cmsk[:, :])

        proj_ctx = ExitStack()
        proj_in = proj_ctx.enter_context(tc.tile_pool(name="proj_in", bufs=1))
        pjps = proj_ctx.enter_context(tc.tile_pool(name="pjps", bufs=2, space="PSUM"))
        xt = [proj_in.tile([128, S], F16, name=f"xt{k}") for k in range(8)]
        wqs = [proj_in.tile([128, 2 * HL * DH], F16, name=f"wqs{k}") for k in range(8)]
        wvs = [proj_in.tile([128, HL * DH], F16, name=f"wvs{k}") for k in range(8)]
        for k in range(8):
            nc.sync.dma_start(out=xt[k], in_=xT[k * 128:(k + 1) * 128, :])
            nc.sync.dma_start(out=wqs[k], in_=wqk[k * 128:(k + 1) * 128, :])
            nc.sync.dma_start(out=wvs[k], in_=wv[k * 128:(k + 1) * 128, :])

        def proj_v(m):
            ps = pjps.tile([128, 512], F32, tag="pj", name="pjv")
            for k in range(8):
                nc.tensor.matmul(ps, xt[k][:, m * 128:(m + 1) * 128], wvs[k],
                                 start=(k == 0), stop=(k == 7))
            vdst = vsb[m].rearrange("p (h c) -> p h c", c=DH + 1)[:, :, 0:DH]
            nc.vector.tensor_copy(vdst, ps.rearrange("p (h c) -> p h c", c=DH))

        def proj_qk(m):
            for q4 in range(4):
                ps = pjps.tile([128, 512], F32, tag="pj", name="pjqk")
                for k in range(8):
                    nc.tensor.matmul(ps, wqs[k][:, m * 128:(m + 1) * 128],
                                     xt[k][:, q4 * 512:(q4 + 1) * 512],
                                     start=(k == 0), stop=(k == 7))
                nc.vector.tensor_copy(qkt[m][:, q4 * 512:(q4 + 1) * 512], ps)

        # V first (attention consumes all V tiles), then QK in pair order
        for m in range(16):
            proj_v(m)
        for t in range(4):
            proj_qk(t)
            proj_qk(4 + t)
        proj_ctx.close()

        attn_ctx = ExitStack()
        adram = attn_ctx.enter_context(tc.tile_pool(name="adram", bufs=2, space="DRAM"))
        asb = attn_ctx.enter_context(tc.tile_pool(name="asb", bufs=6))
        scps = attn_ctx.enter_context(tc.tile_pool(name="scps", bufs=2, space="PSUM"))
        accps = attn_ctx.enter_context(tc.tile_pool(name="accps", bufs=1, space="PSUM"))
        small = attn_ctx.enter_context(tc.tile_pool(name="small", bufs=2))

        def attn_chunk(t, qc):
            q_t, k_t = qkt[t], qkt[4 + t]
            nkb = 4 * qc + 4
            qsl = slice(qc * 512, (qc + 1) * 512)
            av0 = accps.tile([65, 512], F32, tag="av0", name="av0")
            av1 = accps.tile([65, 512], F32, tag="av1", name="av1")
            w = DH + 1
            for kb in range(nkb):
                ksl = slice(kb * 128, (kb + 1) * 128)
                s0 = scps.tile([128, 512], F32, tag="s0", name="s0")
                s1 = scps.tile([128, 512], F32, tag="s1", name="s1")
                # scores^T = K^T.T @ Q^T, two heads row-packed (K=64 each)
                nc.tensor.matmul(s0, k_t[0:64, ksl], q_t[0:64, qsl],
                                 start=True, stop=True)
                nc.tensor.matmul(s1, k_t[64:128, ksl], q_t[64:128, qsl],
                                 start=True, stop=True)
                e0 = asb.tile([128, 512], F16, tag="e0", name="e0")
                e1 = asb.tile([128, 512], F16, tag="e1", name="e1")
                nc.scalar.activation(e0, s0, Exp, scale=SCALE)
                nc.scalar.activation(e1, s1, Exp, scale=SCALE)
                r = kb - 4 * qc
                if r >= 0:  # diagonal block: zero out masked entries
                    msl = slice(r * 512, (r + 1) * 512)
                    nc.vector.tensor_mul(e0, e0, cm[:, msl])
                    nc.vector.tensor_mul(e1, e1, cm[:, msl])
                st, sp = (kb == 0), (kb == nkb - 1)
                # attn-out^T accumulation; V carries a trailing ones column,
                # so the softmax denominator accumulates into PSUM row 64
                nc.tensor.matmul(av0, vsb[kb][:, (2 * t) * w:(2 * t + 1) * w],
                                 e0, start=st, stop=sp)
                nc.tensor.matmul(av1, vsb[kb][:, (2 * t + 1) * w:(2 * t + 2) * w],
                                 e1, start=st, stop=sp)
            # drain PSUM quickly (DVE), then normalize via DRAM-roundtrip
            # broadcast + gpsimd divide, all off the PE/DVE critical path
            nc.vector.tensor_copy(aot_u[t][0:64, qsl], av0[0:64, :])
            stg = small.tile([64, 512], F32, tag="stg", name="stg", bufs=4)
            nc.vector.tensor_copy(stg, av1[0:64, :])
            nc.sync.dma_start(out=aot_u[t][64:128, qsl], in_=stg)
            d0 = small.tile([65, 512], F32, tag="d0", name="d0")
            d1 = small.tile([65, 512], F32, tag="d1", name="d1")
            nc.vector.tensor_copy(d0[64:65, :], av0[64:65, :])
            nc.vector.tensor_copy(d1[64:65, :], av1[64:65, :])
            rdend = adram.tile([2, 512], F32, tag="rdend", name="rdend")
            nc.sync.dma_start(out=rdend[0:1, :], in_=d0[64:65, :])
            nc.sync.dma_start(out=rdend[1:2, :], in_=d1[64:65, :])
            rb = small.tile([128, 512], F32, tag="rb", name="rb")
            for jh in range(2):
                srow = rdend[jh:jh + 1, :]
                bc = bass.AP(tensor=srow.tensor, offset=srow.offset,
                             ap=[[0, 64], [1, 512]])
                nc.gpsimd.dma_start(out=rb[jh * 64:(jh + 1) * 64, :], in_=bc)
            # deferred (low priority): the reciprocal is slow on DVE, let it
            # fill DVE idle slots instead of blocking the attention pipeline
            with tc.high_priority(offset=-800):
                nc.vector.reciprocal(rb, rb)
                nc.vector.tensor_mul(aot[t][:, qsl], aot_u[t][:, qsl], rb)

        fin = ExitStack()
        dpool = fin.enter_context(tc.tile_pool(name="dram", bufs=1, space="DRAM"))
        fsb = fin.enter_context(tc.tile_pool(name="fsb", bufs=1))
        fps = fin.enter_context(tc.tile_pool(name="fps", bufs=2, space="PSUM"))
        lnp = fin.enter_context(tc.tile_pool(name="lnp", bufs=2))

        wos = [fsb.tile([128, D], F16, name=f"wos{k}") for k in range(4)]
        xr = [fsb.tile([128, D], F32, name=f"xr{k}") for k in range(8)]
        for k in range(4):
            nc.sync.dma_start(out=wos[k], in_=wo[k * 128:(k + 1) * 128, :])
        for k in range(8):
            nc.sync.dma_start(out=xr[k], in_=xres[k * 128:(k + 1) * 128, :])

        # chunked fp16 ReduceScatter over the pair, overlapped with both the
        # output projection and the remaining attention sweep: chunk c
        # carries output rows [c*256, (c+1)*256) of each query half.
        rs_in = [dpool.tile([512, D], F16, name=f"rs_in{c}", bufs=4) for c in range(4)]
        rs_out = [dpool.tile([256, D], F16, name=f"rs_out{c}", bufs=4) for c in range(4)]

        def out_chunk(c):
            for j, m in enumerate((2 * c, 2 * c + 1, 8 + 2 * c, 8 + 2 * c + 1)):
                pstg = lnp.tile([128, D], F16, tag="pstg", name="pstg")
                for n2 in range(2):
                    po = fps.tile([128, 512], F32, tag="po", name="po")
                    for k in range(4):
                        nc.tensor.matmul(po, aot[k][:, m * 128:(m + 1) * 128],
                                         wos[k][:, n2 * 512:(n2 + 1) * 512],
                                         start=(k == 0), stop=(k == 3))
                    nc.vector.tensor_copy(pstg[:, n2 * 512:(n2 + 1) * 512], po)
                nc.sync.dma_start(out=rs_in[c][j * 128:(j + 1) * 128, :], in_=pstg)
            nc.gpsimd.collective_compute(
                "ReduceScatter", mybir.AluOpType.add,
                replica_groups=[[0, 1], [2, 3], [4, 5], [6, 7]],
                ins=[rs_in[c].opt()], outs=[rs_out[c].opt()])

        def ln_chunk(c):
            for j in range(2):
                m = 2 * c + j
                y = lnp.tile([128, D], F32, tag="y", name="y")
                yin = lnp.tile([128, D], F16, tag="yin", name="yin")
                nc.gpsimd.dma_start(out=yin, in_=rs_out[c][j * 128:(j + 1) * 128, :])
                nc.vector.tensor_add(y, yin, xr[m])
                stats = lnp.tile([128, 2, 6], F32, tag="st", name="st")
                mv = lnp.tile([128, 2], F32, tag="mv", name="mv")
                for sg in range(2):
                    nc.vector.bn_stats(out=stats[:, sg, :], in_=y[:, sg * 512:(sg + 1) * 512])
                nc.vector.bn_aggr(out=mv, in_=stats)
                rstd = lnp.tile([128, 1], F32, tag="rs", name="rs")
                nc.scalar.activation(out=rstd, in_=mv[:, 1:2], func=Sqrt, bias=eps_t)
                nc.vector.reciprocal(rstd, rstd)
                ot = lnp.tile([128, D], F32, tag="ot", name="ot")
                nc.vector.tensor_scalar(out=ot, in0=y, scalar1=mv[:, 0:1], scalar2=rstd,
                                        op0=mybir.AluOpType.subtract,
                                        op1=mybir.AluOpType.mult)
                nc.sync.dma_start(out=out[m * 128:(m + 1) * 128, :], in_=ot)

        # first attention sweep covers q in [0,1024) and [1024,1536) halves:
        # qc 0 and 2 complete output chunks 0 and 1
        for qc in (0, 2):
            for t in range(4):
                attn_chunk(t, qc)
        out_chunk(0)
        ln_chunk(0)
        for t in range(4):
            attn_chunk(t, 1)
        out_chunk(1)
        ln_chunk(1)
        for t in range(4):
            attn_chunk(t, 3)
        out_chunk(2)
        ln_chunk(2)
        out_chunk(3)
        ln_chunk(3)
        fin.close()
        attn_ctx.close()
    nc.compile()
    return nc


def _build_cmask():
    k = np.arange(128)[:, None]
    q = np.arange(512)[None, :]
    blocks = [(r * 128 + k <= q).astype(np.float16) for r in range(4)]
    return np.concatenate(blocks, axis=1)


def _make_in_maps(x0, W_in, W_o):
    x0 = np.asarray(x0, np.float32)
    W_in = np.asarray(W_in, np.float32)
    W_o = np.asarray(W_o, np.float32)
    wo16 = W_o.astype(np.float16)
    cmask = _build_cmask()
    in_maps = []
    for core in range(NCORES):
        bi, half = core // 2, core % 2
        hs = range(half * HL, half * HL + HL)
        wqk = np.concatenate(
            [W_in[:, h * 3 * DH: h * 3 * DH + DH] for h in hs]
            + [W_in[:, h * 3 * DH + DH: h * 3 * DH + 2 * DH] for h in hs], axis=1)
        wv = np.concatenate(
            [W_in[:, h * 3 * DH + 2 * DH: h * 3 * DH + 3 * DH] for h in hs], axis=1)
        in_maps.append(dict(
            xT=np.ascontiguousarray(x0[bi].T).astype(np.float16),
            wqk=np.ascontiguousarray(wqk).astype(np.float16),
            wv=np.ascontiguousarray(wv).astype(np.float16),
            wo=np.ascontiguousarray(wo16[half * HL * DH:(half + 1) * HL * DH]),
            xres=np.ascontiguousarray(x0[bi, half * SH:(half + 1) * SH]),
            cmask=cmask))
    return in_maps


_NC = None


def _run(x0, W_in, W_o, **run_kwargs):
    global _NC
    if _NC is None:
        _NC = build_nc()
    in_maps = _make_in_maps(x0, W_in, W_o)
    return run_bass_kernel_spmd(_NC, in_maps, list(range(NCORES)), **run_kwargs)


def kernel(x0, W_in, W_o, src_mask=None):
    res = _run(x0, W_in, W_o).results
    out = np.empty((B, S, D), np.float32)
    for core in range(NCORES):
        bi, half = core // 2, core % 2
        out[bi, half * SH:(half + 1) * SH] = res[core]["out"]
    return out



# revision 4
# speedup vs baseline: 1.1009x; 1.1009x over previous
"""Trainium2 Bass kernel for a causal dense-transformer attention layer.

Reference computation (b=4, s=2048, d=1024, 16 heads, dh=64):
  qkv = x0 @ W_in ; causal softmax attention ; out = attn @ W_o
  y = LayerNorm(out + x0)   (no affine, eps=1e-5)

Sharding over 8 cores: core = (batch bi = core//2, head-group tp = core%2).
Each core computes QKV projection + attention for its 8 heads of one batch
(tensor parallel over head groups), then the output projection partial sums
are pair-ReduceScattered so residual + LayerNorm run locally on each core's
1024 output rows.

v2 layout notes:
- scores are computed transposed (keys on partitions, queries free); both
  heads of a pair write adjacent PSUM banks of one [128,1024] tile so a
  single ACT exp instruction covers them (the Scalar engine is the pacing
  resource in the attention phase).
- softmax denominators ride as a ones-column inside V (PSUM row 64); they
  are normalized once per query-chunk: one batched reciprocal on an [8,512]
  gather, broadcast back via DRAM-roundtrip gpsimd DMAs.
- program order starts attention for head-pair 0 right after its Q/K
  projection so exp starts ~25us in; out-projection blocks are issued per
  128-row block as soon as their query-chunk is normalized, leaving only
  two small ReduceScatters exposed at the end.
"""

import os
import sys
from contextlib import ExitStack

import numpy as np

for _p in ("/opt/trn_rl_repo",):
    if os.path.isdir(_p) and _p not in sys.path:
        sys.path.insert(0, _p)

import concourse.bass as bass
import concourse.tile as tile
from concourse import bacc
from concourse import mybir
from concourse.bass_utils import run_bass_kernel_spmd

B, S, D = 4, 2048, 1024
NH, DH = 16, 64
HL = NH // 2          # heads per core
SH = S // 2           # output seq rows per core
NCORES = 8
SCALE = DH ** -0.5    # 0.125
LN_EPS = 1e-5

F16 = mybir.dt.float16
F32 = mybir.dt.float32
Exp = mybir.ActivationFunctionType.Exp
Sqrt = mybir.ActivationFunctionType.Sqrt

# out-proj chunk c -> the four 128-row q blocks it carries (2 low, 2 high)
CHUNKS = [[0, 1, 8, 9], [2, 3, 10, 11], [4, 5, 12, 13], [6, 7, 14, 15]]


def build_nc():
    nc = bacc.Bacc("TRN2", target_bir_lowering=False, num_devices=NCORES)
    xT = nc.declare_dram_parameter("xT", [D, S], F16, isOutput=False)
    wqk = nc.declare_dram_parameter("wqk", [D, 2 * HL * DH], F16, isOutput=False)
    wv = nc.declare_dram_parameter("wv", [D, HL * DH], F16, isOutput=False)
    wo = nc.declare_dram_parameter("wo", [HL * DH, D], F16, isOutput=False)
    xres = nc.declare_dram_parameter("xres", [SH, D], F32, isOutput=False)
    cmsk = nc.declare_dram_parameter("cmask", [128, 4 * 1024], F16, isOutput=False)
    out = nc.declare_dram_parameter("out", [SH, D], F32, isOutput=True)

    with tile.TileContext(nc, num_cores=NCORES) as tc, ExitStack() as top:
        persist = top.enter_context(tc.tile_pool(name="persist", bufs=1))
        # QT rows 0..511 (tiles 0-3, head pair t on tile t), KT rows 512..1023
        qkt = [persist.tile([128, S], F16, name=f"qkt{m}") for m in range(8)]
        # V in (seq-part, head*dh free) orientation + trailing ones column
        vsb = [persist.tile([128, HL * (DH + 1)], F16, name=f"vsb{m}") for m in range(16)]
        # normalized attn-out^T (head*dh on partitions, seq free)
        aot = [persist.tile([128, S], F16, name=f"aot{t}") for t in range(4)]
        cm = persist.tile([128, 4 * 1024], F16, name="cm")
        eps_t = persist.tile([128, 1], F32, name="eps_t")
        nc.vector.memset(eps_t, LN_EPS)
        for m in range(16):
            vones = vsb[m].rearrange("p (h c) -> p h c", c=DH + 1)[:, :, DH:DH + 1]
            nc.vector.memset(vones, 1.0)
        nc.sync.dma_start(out=cm, in_=cmsk[:, :])

        # attention pools open first so their PSUM banks never alias the
        # projection PSUM pool (pools are a strict stack; proj closes first)
        attn_ctx = ExitStack()
        adram = attn_ctx.enter_context(tc.tile_pool(name="adram", bufs=2, space="DRAM"))
        asb = attn_ctx.enter_context(tc.tile_pool(name="asb", bufs=5))
        scps = attn_ctx.enter_context(tc.tile_pool(name="scps", bufs=2, space="PSUM"))
        avps = attn_ctx.enter_context(tc.tile_pool(name="avps", bufs=1, space="PSUM"))
        small = attn_ctx.enter_context(tc.tile_pool(name="small", bufs=2))

        proj_ctx = ExitStack()
        proj_in = proj_ctx.enter_context(tc.tile_pool(name="proj_in", bufs=1))
        pjps = proj_ctx.enter_context(tc.tile_pool(name="pjps", bufs=2, space="PSUM"))
        xt = [proj_in.tile([128, S], F16, name=f"xt{k}") for k in range(8)]
        wqs = [proj_in.tile([128, 2 * HL * DH], F16, name=f"wqs{k}") for k in range(8)]
        wvs = [proj_in.tile([128, HL * DH], F16, name=f"wvs{k}") for k in range(8)]
        # interleave so the k-accumulation stream can start on first arrivals
        for k in range(8):
            nc.sync.dma_start(out=xt[k], in_=xT[k * 128:(k + 1) * 128, :])
            nc.sync.dma_start(out=wqs[k], in_=wqk[k * 128:(k + 1) * 128, :])
        for k in range(8):
            nc.sync.dma_start(out=wvs[k], in_=wv[k * 128:(k + 1) * 128, :])

        def proj_v(m):
            ps = pjps.tile([128, 512], F32, tag="pj", name="pjv")
            for k in range(8):
                nc.tensor.matmul(ps, xt[k][:, m * 128:(m + 1) * 128], wvs[k],
                                 start=(k == 0), stop=(k == 7))
            vdst = vsb[m].rearrange("p (h c) -> p h c", c=DH + 1)[:, :, 0:DH]
            nc.vector.tensor_copy(vdst, ps.rearrange("p (h c) -> p h c", c=DH))

        def proj_qk(m):
            for q4 in range(4):
                ps = pjps.tile([128, 512], F32, tag="pj", name="pjqk")
                for k in range(8):
                    nc.tensor.matmul(ps, wqs[k][:, m * 128:(m + 1) * 128],
                                     xt[k][:, q4 * 512:(q4 + 1) * 512],
                                     start=(k == 0), stop=(k == 7))
                nc.vector.tensor_copy(qkt[m][:, q4 * 512:(q4 + 1) * 512], ps)

        # Q/K for pair t, then its V needs, staged so attention t0/qc0 can
        # begin while the rest of the projection streams on the PE
        proj_qk(0); proj_qk(4)
        proj_v(0); proj_v(1); proj_v(2); proj_v(3)
        proj_qk(1); proj_qk(5)
        proj_v(4); proj_v(5); proj_v(6); proj_v(7)
        proj_qk(2); proj_qk(6)
        proj_v(8); proj_v(9); proj_v(10); proj_v(11)
        proj_qk(3); proj_qk(7)
        proj_v(12); proj_v(13); proj_v(14); proj_v(15)

        w = DH + 1

        def attn_chunk(t, qc, qdn):
            q_t, k_t = qkt[t], qkt[4 + t]
            nkb = 4 * qc + 4
            qsl = slice(qc * 512, (qc + 1) * 512)
            av = avps.tile([65, 1024], F32, tag="av", name="av")
            for kb in range(nkb):
                ksl = slice(kb * 128, (kb + 1) * 128)
                sp = scps.tile([128, 1024], F32, tag="sp", name="sp")
                # scores^T = K^T.T @ Q^T; the two heads land in adjacent
                # PSUM banks and use disjoint PE row groups (h0 / h64)
                nc.tensor.matmul(sp[:, 0:512], k_t[0:64, ksl], q_t[0:64, qsl],
                                 start=True, stop=True)
                nc.tensor.matmul(sp[:, 512:1024], k_t[64:128, ksl], q_t[64:128, qsl],
                                 start=True, stop=True)
                ep = asb.tile([128, 1024], F16, tag="ep", name="ep", bufs=5)
                nc.scalar.activation(ep, sp, Exp, scale=SCALE)
                r = kb - 4 * qc
                if r >= 0:  # diagonal block: zero masked entries (exact)
                    nc.vector.tensor_mul(ep, ep, cm[:, r * 1024:(r + 1) * 1024])
                st, sp_ = (kb == 0), (kb == nkb - 1)
                # attn-out^T accumulation; V carries a trailing ones column,
                # so the softmax denominator accumulates into PSUM row 64
                nc.tensor.matmul(av[:, 0:512], vsb[kb][:, (2 * t) * w:(2 * t + 1) * w],
                                 ep[:, 0:512], start=st, stop=sp_)
                nc.tensor.matmul(av[:, 512:1024], vsb[kb][:, (2 * t + 1) * w:(2 * t + 2) * w],
                                 ep[:, 512:1024], start=st, stop=sp_)
            # drain PSUM: rows 0..63 data, row 64 denominator
            au = small.tile([128, 512], F32, tag="au", name="au", bufs=6)
            stg = small.tile([65, 512], F32, tag="stg", name="stg", bufs=3)
            nc.vector.tensor_copy(au[0:65, :], av[:, 0:512])
            nc.vector.tensor_copy(stg, av[:, 512:1024])
            nc.sync.dma_start(out=qdn[2 * t:2 * t + 1, :], in_=au[64:65, :])
            nc.sync.dma_start(out=qdn[2 * t + 1:2 * t + 2, :], in_=stg[64:65, :])
            nc.sync.dma_start(out=au[64:128, :], in_=stg[0:64, :])
            return au

        def norm_qc(qc, qdn, aus):
            # one reciprocal for all 8 denominator rows of this query chunk,
            # then broadcast across partitions via DRAM-roundtrip DMA
            dn = small.tile([8, 512], F32, tag="dn", name="dn", bufs=2)
            nc.gpsimd.dma_start(out=dn, in_=qdn[:, :])
            nc.vector.reciprocal(dn, dn)
            rdn = adram.tile([8, 512], F32, tag="rdn", name="rdn", bufs=2)
            nc.sync.dma_start(out=rdn, in_=dn)
            qsl = slice(qc * 512, (qc + 1) * 512)
            for t in range(4):
                rb = small.tile([128, 512], F32, tag="rb", name="rb", bufs=2)
                for j in range(2):
                    srow = rdn[2 * t + j:2 * t + j + 1, :]
                    bc = bass.AP(tensor=srow.tensor, offset=srow.offset,
                                 ap=[[0, 64], [1, 512]])
                    nc.gpsimd.dma_start(out=rb[j * 64:(j + 1) * 64, :], in_=bc)
                with tc.high_priority(offset=-400):
                    nc.vector.tensor_mul(aot[t][:, qsl], aus[t], rb)

        def run_qc(qc):
            qdn = adram.tile([8, 512], F32, tag="qdn", name="qdn", bufs=2)
            aus = [attn_chunk(t, qc, qdn) for t in range(4)]
            norm_qc(qc, qdn, aus)

        run_qc(0)
        run_qc(1)
        proj_ctx.close()

        fin = ExitStack()
        dpool = fin.enter_context(tc.tile_pool(name="dram", bufs=1, space="DRAM"))
        fsb = fin.enter_context(tc.tile_pool(name="fsb", bufs=1))
        fps = fin.enter_context(tc.tile_pool(name="fps", bufs=2, space="PSUM"))
        lnp = fin.enter_context(tc.tile_pool(name="lnp", bufs=2))

        wos = [fsb.tile([128, D], F16, name=f"wos{k}") for k in range(4)]
        for k in range(4):
            nc.sync.dma_start(out=wos[k], in_=wo[k * 128:(k + 1) * 128, :])

        rs_in = [dpool.tile([512, D], F16, name=f"rs_in{c}", bufs=4) for c in range(4)]
        rs_out = [dpool.tile([256, D], F16, name=f"rs_out{c}", bufs=4) for c in range(4)]

        def out_j(c, j):
            m = CHUNKS[c][j]
            pstg = lnp.tile([128, D], F16, tag="pstg", name="pstg")
            for n2 in range(2):
                po = fps.tile([128, 512], F32, tag="po", name="po")
                for k in range(4):
                    nc.tensor.matmul(po, aot[k][:, m * 128:(m + 1) * 128],
                                     wos[k][:, n2 * 512:(n2 + 1) * 512],
                                     start=(k == 0), stop=(k == 3))
                nc.vector.tensor_copy(pstg[:, n2 * 512:(n2 + 1) * 512], po)
            nc.sync.dma_start(out=rs_in[c][j * 128:(j + 1) * 128, :], in_=pstg)

        def rs_c(c):
            nc.gpsimd.collective_compute(
                "ReduceScatter", mybir.AluOpType.add,
                replica_groups=[[0, 1], [2, 3], [4, 5], [6, 7]],
                ins=[rs_in[c].opt()], outs=[rs_out[c].opt()])

        def ln_c(c):
            for j in range(2):
                m = 2 * c + j
                y = lnp.tile([128, D], F32, tag="y", name="y")
                yin = lnp.tile([128, D], F16, tag="yin", name="yin")
                xr = lnp.tile([128, D], F32, tag="xr", name="xr")
                nc.sync.dma_start(out=xr, in_=xres[m * 128:(m + 1) * 128, :])
                nc.gpsimd.dma_start(out=yin, in_=rs_out[c][j * 128:(j + 1) * 128, :])
                nc.vector.tensor_add(y, yin, xr)
                stats = lnp.tile([128, 2, 6], F32, tag="st", name="st")
                mv = lnp.tile([128, 2], F32, tag="mv", name="mv")
                for sg in range(2):
                    nc.vector.bn_stats(out=stats[:, sg, :], in_=y[:, sg * 512:(sg + 1) * 512])
                nc.vector.bn_aggr(out=mv, in_=stats)
                rstd = lnp.tile([128, 1], F32, tag="rs", name="rs")
                nc.scalar.activation(out=rstd, in_=mv[:, 1:2], func=Sqrt, bias=eps_t)
                nc.vector.reciprocal(rstd, rstd)
                ot = lnp.tile([128, D], F32, tag="ot", name="ot")
                nc.vector.tensor_scalar(out=ot, in0=y, scalar1=mv[:, 0:1], scalar2=rstd,
                                        op0=mybir.AluOpType.subtract,
                                        op1=mybir.AluOpType.mult)
                nc.sync.dma_start(out=out[m * 128:(m + 1) * 128, :], in_=ot)

        # q blocks 0..7 (chunk lows) can project as soon as qc0/qc1 land
        for c in range(4):
            out_j(c, 0)
            out_j(c, 1)
        run_qc(2)
        out_j(0, 2); out_j(0, 3)
        rs_c(0); ln_c(0)
        out_j(1, 2); out_j(1, 3)
        rs_c(1); ln_c(1)
        run_qc(3)
        out_j(2, 2); out_j(2, 3)
        rs_c(2); ln_c(2)
        out_j(3, 2); out_j(3, 3)
        rs_c(3); ln_c(3)
        fin.close()
        attn_ctx.close()
    nc.compile()
    return nc


def _build_cmask():
    # block r: [128,512] causal pattern (r*128 + k <= q), duplicated for the
    # two packed heads -> [128, 1024] per r, 4 r blocks side by side
    k = np.arange(128)[:, None]
    q = np.arange(512)[None, :]
    blocks = []
    for r in range(4):
        m = (r * 128 + k <= q).astype(np.float16)
        blocks.append(np.concatenate([m, m], axis=1))
    return np.concatenate(blocks, axis=1)


def _make_in_maps(x0, W_in, W_o):
    x0 = np.asarray(x0, np.float32)
    W_in = np.asarray(W_in, np.float32)
    W_o = np.asarray(W_o, np.float32)
    wo16 = W_o.astype(np.float16)
    cmask = _build_cmask()
    in_maps = []
    for core in range(NCORES):
        bi, half = core // 2, core % 2
        hs = range(half * HL, half * HL + HL)
        wqk = np.concatenate(
            [W_in[:, h * 3 * DH: h * 3 * DH + DH] for h in hs]
            + [W_in[:, h * 3 * DH + DH: h * 3 * DH + 2 * DH] for h in hs], axis=1)
        wv = np.concatenate(
            [W_in[:, h * 3 * DH + 2 * DH: h * 3 * DH + 3 * DH] for h in hs], axis=1)
        in_maps.append(dict(
            xT=np.ascontiguousarray(x0[bi].T).astype(np.float16),
            wqk=np.ascontiguousarray(wqk).astype(np.float16),
            wv=np.ascontiguousarray(wv).astype(np.float16),
            wo=np.ascontiguousarray(wo16[half * HL * DH:(half + 1) * HL * DH]),
            xres=np.ascontiguousarray(x0[bi, half * SH:(half + 1) * SH]),
            cmask=cmask))
    return in_maps


_NC = None


def _run(x0, W_in, W_o, **run_kwargs):
    global _NC
    if _NC is None:
        _NC = build_nc()
    in_maps = _make_in_maps(x0, W_in, W_o)
    return run_bass_kernel_spmd(_NC, in_maps, list(range(NCORES)), **run_kwargs)


def kernel(x0, W_in, W_o, src_mask=None):
    res = _run(x0, W_in, W_o).results
    out = np.empty((B, S, D), np.float32)
    for core in range(NCORES):
        bi, half = core // 2, core % 2
        out[bi, half * SH:(half + 1) * SH] = res[core]["out"]
    return out


# revision 16
# speedup vs baseline: 1.1698x; 1.0626x over previous
"""Trainium2 Bass kernel for a causal dense-transformer attention layer.

Reference computation (b=4, s=2048, d=1024, 16 heads, dh=64):
  qkv = x0 @ W_in ; causal softmax attention ; out = attn @ W_o
  y = LayerNorm(out + x0)   (no affine, eps=1e-5)

Sharding over 8 cores: core = (batch bi = core//2, head-group tp = core%2).
Each core computes QKV projection + attention for its 8 heads of one batch
(tensor parallel over head groups), then the output projection partial sums
are pair-ReduceScattered so residual + LayerNorm run locally on each core's
1024 output rows.

v2 layout notes:
- scores are computed transposed (keys on partitions, queries free); both
  heads of a pair write adjacent PSUM banks of one [128,1024] tile so a
  single ACT exp instruction covers them (the Scalar engine is the pacing
  resource in the attention phase).
- softmax denominators ride as a ones-column inside V (PSUM row 64); they
  are normalized once per query-chunk: one batched reciprocal on an [8,512]
  gather, broadcast back via DRAM-roundtrip gpsimd DMAs.
- program order starts attention for head-pair 0 right after its Q/K
  projection so exp starts ~25us in; out-projection blocks are issued per
  128-row block as soon as their query-chunk is normalized, leaving only
  two small ReduceScatters exposed at the end.
"""

import os
import sys
from contextlib import ExitStack

import numpy as np

for _p in ("/opt/trn_rl_repo",):
    if os.path.isdir(_p) and _p not in sys.path:
        sys.path.insert(0, _p)

import concourse.bass as bass
import concourse.tile as tile
from concourse import bacc
from concourse import mybir
from concourse.bass_utils import run_bass_kernel_spmd

B, S, D = 4, 2048, 1024
NH, DH = 16, 64
HL = NH // 2          # heads per core
SH = S // 2           # output seq rows per core
NCORES = 8
SCALE = DH ** -0.5    # 0.125
LN_EPS = 1e-5

F16 = mybir.dt.float16
F32 = mybir.dt.float32
Exp = mybir.ActivationFunctionType.Exp
Ln = mybir.ActivationFunctionType.Ln

# out-proj chunk c -> the four 128-row q blocks it carries (2 low, 2 high)
CHUNKS = [[0, 1, 8, 9], [2, 3, 10, 11], [4, 5, 12, 13], [6, 7, 14, 15]]


def build_nc():
    nc = bacc.Bacc("TRN2", target_bir_lowering=False, num_devices=NCORES)
    xT = nc.declare_dram_parameter("xT", [D, S], F16, isOutput=False)
    wqk = nc.declare_dram_parameter("wqk", [D, 2 * HL * DH], F16, isOutput=False)
    wv = nc.declare_dram_parameter("wv", [D, HL * DH], F16, isOutput=False)
    wo = nc.declare_dram_parameter("wo", [HL * DH, D], F16, isOutput=False)
    xres = nc.declare_dram_parameter("xres", [SH, D], F32, isOutput=False)
    # [T | T]: the 128x128 causal triangle (k<=q), duplicated for both heads
    cmsk = nc.declare_dram_parameter("cmask", [128, 256], F16, isOutput=False)
    out = nc.declare_dram_parameter("out", [SH, D], F32, isOutput=True)

    with tile.TileContext(nc, num_cores=NCORES) as tc, ExitStack() as top:
        persist = top.enter_context(tc.tile_pool(name="persist", bufs=1))
        # QT rows 0..511 (tiles 0-3, head pair t on tile t), KT rows 512..1023
        qkt = [persist.tile([128, S], F16, name=f"qkt{m}") for m in range(8)]
        # V in (seq-part, head*dh free) orientation + trailing ones column
        vsb = [persist.tile([128, HL * (DH + 1)], F16, name=f"vsb{m}") for m in range(16)]
        # normalized attn-out^T (head*dh on partitions, seq free)
        aot = [persist.tile([128, S], F16, name=f"aot{t}") for t in range(4)]
        cm = persist.tile([128, 256], F16, name="cm")
        eps_t = persist.tile([128, 1], F32, name="eps_t")
        nc.vector.memset(eps_t, LN_EPS)
        for m in range(16):
            vones = vsb[m].rearrange("p (h c) -> p h c", c=DH + 1)[:, :, DH:DH + 1]
            nc.vector.memset(vones, 1.0)
        nc.sync.dma_start(out=cm, in_=cmsk[:, :])

        # attention pools open first so their PSUM banks never alias the
        # projection PSUM pool (pools are a strict stack; proj closes first)
        attn_ctx = ExitStack()
        adram = attn_ctx.enter_context(tc.tile_pool(name="adram", bufs=2, space="DRAM"))
        asb = attn_ctx.enter_context(tc.tile_pool(name="asb", bufs=5))
        scps = attn_ctx.enter_context(tc.tile_pool(name="scps", bufs=2, space="PSUM"))
        avps = attn_ctx.enter_context(tc.tile_pool(name="avps", bufs=1, space="PSUM"))
        small = attn_ctx.enter_context(tc.tile_pool(name="small", bufs=2))

        proj_ctx = ExitStack()
        proj_in = proj_ctx.enter_context(tc.tile_pool(name="proj_in", bufs=1))
        pjps = proj_ctx.enter_context(tc.tile_pool(name="pjps", bufs=2, space="PSUM"))
        xt = [proj_in.tile([128, S], F16, name=f"xt{k}") for k in range(8)]
        wqs = [proj_in.tile([128, 2 * HL * DH], F16, name=f"wqs{k}") for k in range(8)]
        wvs = [proj_in.tile([128, HL * DH], F16, name=f"wvs{k}") for k in range(8)]
        # interleave so the k-accumulation stream can start on first arrivals
        for k in range(8):
            nc.sync.dma_start(out=xt[k], in_=xT[k * 128:(k + 1) * 128, :])
            nc.sync.dma_start(out=wqs[k], in_=wqk[k * 128:(k + 1) * 128, :])
        for k in range(8):
            nc.sync.dma_start(out=wvs[k], in_=wv[k * 128:(k + 1) * 128, :])

        def proj_v(m):
            ps = pjps.tile([128, 512], F32, tag="pj", name="pjv")
            for k in range(8):
                nc.tensor.matmul(ps, xt[k][:, m * 128:(m + 1) * 128], wvs[k],
                                 start=(k == 0), stop=(k == 7))
            vdst = vsb[m].rearrange("p (h c) -> p h c", c=DH + 1)[:, :, 0:DH]
            nc.vector.tensor_copy(vdst, ps.rearrange("p (h c) -> p h c", c=DH))

        def proj_qk(m):
            for q4 in range(4):
                ps = pjps.tile([128, 512], F32, tag="pj", name="pjqk")
                for k in range(8):
                    nc.tensor.matmul(ps, wqs[k][:, m * 128:(m + 1) * 128],
                                     xt[k][:, q4 * 512:(q4 + 1) * 512],
                                     start=(k == 0), stop=(k == 7))
                nc.vector.tensor_copy(qkt[m][:, q4 * 512:(q4 + 1) * 512], ps)

        # Q/K for pair t, then its V needs, staged so attention t0/qc0 can
        # begin while the rest of the projection streams on the PE
        proj_qk(0); proj_qk(4)
        proj_v(0); proj_v(1); proj_v(2); proj_v(3)
        proj_qk(1); proj_qk(5)
        proj_v(4); proj_v(5); proj_v(6); proj_v(7)
        proj_qk(2); proj_qk(6)
        proj_v(8); proj_v(9); proj_v(10); proj_v(11)
        proj_qk(3); proj_qk(7)
        proj_v(12); proj_v(13); proj_v(14); proj_v(15)

        w = DH + 1

        cmh = cm.rearrange("p (h c) -> p h c", h=2)

        def attn_chunk(t, qlo, qw, nkb, qdn):
            q_t, k_t = qkt[t], qkt[4 + t]
            dlo = qlo // 128  # first diagonal key block index
            HP = 512          # head pitch: keeps all matmul PSUM dsts bank-aligned
            av = avps.tile([65, 2 * HP], F32, tag="av", name="av")
            for kb in range(nkb):
                ksl = slice(kb * 128, (kb + 1) * 128)
                r = kb - dlo
                v0 = 128 * r if r > 0 else 0  # first causally-valid column
                vw = qw - v0
                sp = scps.tile([128, 2 * HP], F32, tag="sp", name="sp")
                # scores^T = K^T.T @ Q^T; the two heads land in adjacent
                # PSUM banks and use disjoint PE row groups (h0 / h64);
                # diagonal blocks write only the valid columns, compacted to
                # the bank start (matmul PSUM dsts must be bank-aligned)
                nc.tensor.matmul(sp[:, 0:vw], k_t[0:64, ksl],
                                 q_t[0:64, qlo + v0:qlo + qw], start=True, stop=True)
                nc.tensor.matmul(sp[:, HP:HP + vw], k_t[64:128, ksl],
                                 q_t[64:128, qlo + v0:qlo + qw], start=True, stop=True)
                ep = asb.tile([128, 2 * HP], F16, tag="ep", name="ep", bufs=5)
                eph = ep.rearrange("p (h q) -> p h q", h=2)
                sph = sp.rearrange("p (h q) -> p h q", h=2)
                # exp de-compacts: reads [0:vw], writes at [v0:qw]
                nc.scalar.activation(eph[:, :, v0:qw], sph[:, :, 0:vw], Exp, scale=SCALE)
                if r >= 0:  # diagonal 128-col block: apply causal triangle
                    if v0 > 0:
                        nc.vector.memset(eph[:, :, 0:v0], 0.0)
                    nc.vector.tensor_mul(eph[:, :, v0:v0 + 128],
                                         eph[:, :, v0:v0 + 128], cmh)
                st, sp_ = (kb == 0), (kb == nkb - 1)
                # attn-out^T accumulation; V carries a trailing ones column,
                # so the softmax denominator accumulates into PSUM row 64
                nc.tensor.matmul(av[:, 0:qw], vsb[kb][:, (2 * t) * w:(2 * t + 1) * w],
                                 ep[:, 0:qw], start=st, stop=sp_)
                nc.tensor.matmul(av[:, HP:HP + qw],
                                 vsb[kb][:, (2 * t + 1) * w:(2 * t + 2) * w],
                                 ep[:, HP:HP + qw], start=st, stop=sp_)
            # drain PSUM: rows 0..63 data, row 64 denominator
            au = small.tile([128, 512], F32, tag="au", name="au", bufs=6)
            stg = small.tile([65, 512], F32, tag="stg", name="stg", bufs=3)
            nc.vector.tensor_copy(au[0:65, 0:qw], av[:, 0:qw])
            nc.vector.tensor_copy(stg[:, 0:qw], av[:, HP:HP + qw])
            nc.sync.dma_start(out=qdn[2 * t:2 * t + 1, 0:qw], in_=au[64:65, 0:qw])
            nc.sync.dma_start(out=qdn[2 * t + 1:2 * t + 2, 0:qw], in_=stg[64:65, 0:qw])
            nc.sync.dma_start(out=au[64:128, 0:qw], in_=stg[0:64, 0:qw])
            return au

        def norm_qc(qlo, qw, qdn, aus):
            # one reciprocal for all 8 denominator rows of this query chunk,
            # then broadcast across partitions via DRAM-roundtrip DMA
            dn = small.tile([8, 512], F32, tag="dn", name="dn", bufs=2)
            nc.gpsimd.dma_start(out=dn[:, 0:qw], in_=qdn[:, 0:qw])
            nc.vector.reciprocal(dn[:, 0:qw], dn[:, 0:qw])
            rdn = adram.tile([8, 512], F32, tag="rdn", name="rdn", bufs=2)
            nc.sync.dma_start(out=rdn[:, 0:qw], in_=dn[:, 0:qw])
            for t in range(4):
                rb = small.tile([128, 512], F32, tag="rb", name="rb", bufs=2)
                for j in range(2):
                    srow = rdn[2 * t + j:2 * t + j + 1, 0:qw]
                    bc = bass.AP(tensor=srow.tensor, offset=srow.offset,
                                 ap=[[0, 64], [1, qw]])
                    nc.gpsimd.dma_start(out=rb[j * 64:(j + 1) * 64, 0:qw], in_=bc)
                with tc.high_priority(offset=-400):
                    nc.vector.tensor_mul(aot[t][:, qlo:qlo + qw], aus[t][:, 0:qw],
                                         rb[:, 0:qw])

        def run_qc(qlo, qw):
            nkb = (qlo + qw) // 128
            qdn = adram.tile([8, 512], F32, tag="qdn", name="qdn", bufs=2)
            aus = [attn_chunk(t, qlo, qw, nkb, qdn) for t in range(4)]
            norm_qc(qlo, qw, qdn, aus)

        run_qc(0, 512)
        run_qc(512, 512)
        proj_ctx.close()

        fin = ExitStack()
        dpool = fin.enter_context(tc.tile_pool(name="dram", bufs=1, space="DRAM"))
        fsb = fin.enter_context(tc.tile_pool(name="fsb", bufs=1))
        fps = fin.enter_context(tc.tile_pool(name="fps", bufs=2, space="PSUM"))
        lnp = fin.enter_context(tc.tile_pool(name="lnp", bufs=2))

        wos = [fsb.tile([128, D], F16, name=f"wos{k}") for k in range(4)]
        for k in range(4):
            nc.sync.dma_start(out=wos[k], in_=wo[k * 128:(k + 1) * 128, :])

        rs_in = [dpool.tile([512, D], F16, name=f"rs_in{c}", bufs=4) for c in range(4)]
        rs_out = [dpool.tile([256, D], F16, name=f"rs_out{c}", bufs=4) for c in range(4)]

        def out_j(c, j):
            m = CHUNKS[c][j]
            pstg = lnp.tile([128, D], F16, tag="pstg", name="pstg")
            for n2 in range(2):
                po = fps.tile([128, 512], F32, tag="po", name="po")
                for k in range(4):
                    nc.tensor.matmul(po, aot[k][:, m * 128:(m + 1) * 128],
                                     wos[k][:, n2 * 512:(n2 + 1) * 512],
                                     start=(k == 0), stop=(k == 3))
                nc.vector.tensor_copy(pstg[:, n2 * 512:(n2 + 1) * 512], po)
            nc.sync.dma_start(out=rs_in[c][j * 128:(j + 1) * 128, :], in_=pstg)

        def rs_c(c):
            nc.gpsimd.collective_compute(
                "ReduceScatter", mybir.AluOpType.add,
                replica_groups=[[0, 1], [2, 3], [4, 5], [6, 7]],
                ins=[rs_in[c].opt()], outs=[rs_out[c].opt()])

        def ln_c(c):
            for j in range(2):
                m = 2 * c + j
                y = lnp.tile([128, D], F32, tag="y", name="y")
                yin = lnp.tile([128, D], F16, tag="yin", name="yin")
                xr = lnp.tile([128, D], F32, tag="xr", name="xr")
                nc.sync.dma_start(out=xr, in_=xres[m * 128:(m + 1) * 128, :])
                nc.gpsimd.dma_start(out=yin, in_=rs_out[c][j * 128:(j + 1) * 128, :])
                nc.vector.tensor_add(y, yin, xr)
                stats = lnp.tile([128, 2, 6], F32, tag="st", name="st")
                mv = lnp.tile([128, 2], F32, tag="mv", name="mv")
                for sg in range(2):
                    nc.vector.bn_stats(out=stats[:, sg, :], in_=y[:, sg * 512:(sg + 1) * 512])
                nc.vector.bn_aggr(out=mv, in_=stats)
                rstd = lnp.tile([128, 1], F32, tag="rs", name="rs")
                nc.scalar.activation(out=rstd, in_=mv[:, 1:2],
                                     func=mybir.ActivationFunctionType.Sqrt,
                                     bias=eps_t)
                nc.vector.reciprocal(rstd, rstd)
                ot = lnp.tile([128, D], F32, tag="ot", name="ot")
                nc.vector.tensor_scalar(out=ot, in0=y, scalar1=mv[:, 0:1], scalar2=rstd,
                                        op0=mybir.AluOpType.subtract,
                                        op1=mybir.AluOpType.mult)
                nc.sync.dma_start(out=out[m * 128:(m + 1) * 128, :], in_=ot)

        # q blocks 0..7 (chunk lows) can project as soon as qc0/qc1 land
        for c in range(4):
            out_j(c, 0)
            out_j(c, 1)
        run_qc(1024, 512)
        out_j(0, 2); out_j(0, 3)
        rs_c(0); ln_c(0)
        out_j(1, 2); out_j(1, 3)
        rs_c(1); ln_c(1)
        # last query range is split in two so chunk 2's RS overlaps the
        # second half's attention, leaving only chunk 3's RS exposed
        run_qc(1536, 256)
        out_j(2, 2); out_j(2, 3)
        rs_c(2); ln_c(2)
        run_qc(1792, 256)
        out_j(3, 2); out_j(3, 3)
        rs_c(3); ln_c(3)
        fin.close()
        attn_ctx.close()
    nc.compile()
    return nc


def _build_cmask():
    # the 128x128 causal triangle (k <= q), duplicated for the two packed
    # heads -> [128, 256]
    k = np.arange(128)[:, None]
    q = np.arange(128)[None, :]
    m = (k <= q).astype(np.float16)
    return np.concatenate([m, m], axis=1)


def _make_in_maps(x0, W_in, W_o):
    x0 = np.asarray(x0, np.float32)
    W_in = np.asarray(W_in, np.float32)
    W_o = np.asarray(W_o, np.float32)
    wo16 = W_o.astype(np.float16)
    cmask = _build_cmask()
    in_maps = []
    for core in range(NCORES):
        bi, half = core // 2, core % 2
        hs = range(half * HL, half * HL + HL)
        wqk = np.concatenate(
            [W_in[:, h * 3 * DH: h * 3 * DH + DH] for h in hs]
            + [W_in[:, h * 3 * DH + DH: h * 3 * DH + 2 * DH] for h in hs], axis=1)
        wv = np.concatenate(
            [W_in[:, h * 3 * DH + 2 * DH: h * 3 * DH + 3 * DH] for h in hs], axis=1)
        in_maps.append(dict(
            xT=np.ascontiguousarray(x0[bi].T).astype(np.float16),
            wqk=np.ascontiguousarray(wqk).astype(np.float16),
            wv=np.ascontiguousarray(wv).astype(np.float16),
            wo=np.ascontiguousarray(wo16[half * HL * DH:(half + 1) * HL * DH]),
            xres=np.ascontiguousarray(x0[bi, half * SH:(half + 1) * SH]),
            cmask=cmask))
    return in_maps


_NC = None


def _run(x0, W_in, W_o, **run_kwargs):
    global _NC
    if _NC is None:
        _NC = build_nc()
    in_maps = _make_in_maps(x0, W_in, W_o)
    return run_bass_kernel_spmd(_NC, in_maps, list(range(NCORES)), **run_kwargs)


def kernel(x0, W_in, W_o, src_mask=None):
    res = _run(x0, W_in, W_o).results
    out = np.empty((B, S, D), np.float32)
    for core in range(NCORES):
        bi, half = core // 2, core % 2
        out[bi, half * SH:(half + 1) * SH] = res[core]["out"]
    return out
